# revision 1
# baseline (speedup 1.0000x reference)
"""Grouped-experts SwiGLU MoE kernel for Trainium2 (8 NeuronCores).

Expert-parallel: core e computes expert e entirely.
  h = silu(x @ gate) * (x @ down); out = h @ up
v3 = v2 (pre-tiled contiguous DMA layouts, bf16 operands, fp32 PSUM) plus
G=2 stationary sharing: every LDWEIGHTS serves two matmuls (measured
216.8 ns/MM vs 242.4 with LDW-per-MM):
  phase 1: for each k, one gate-weight load feeds both 512-token chunks
           (pg0/pg1 banks); gate chains for the whole k-range run first,
           then down chains (pd0/pd1) — silu(pg*) drains while the down
           chains occupy the PE, so the 4 banks recycle with no stall.
  phase 2: one h-tile load feeds two output-column chunks (po0/po1).
PSUM budget: pg0,pg1,pd0,pd1 (bufs=1) + po0,po1 (bufs=2) = 8 banks.
"""
import sys
if '/opt/trn_rl_repo' not in sys.path:
    sys.path.insert(0, '/opt/trn_rl_repo')
import numpy as np
import ml_dtypes
from concourse import bacc, tile, mybir, bass_utils

E, T, D_IN, D_H = 8, 4096, 2048, 1408
T_B = 1024                 # tokens per block
NK = D_IN // 128           # 16 k-tiles (phase-1 contraction)
NJ = D_H // 128            # 11 j-tiles
NB = T // T_B              # 4 blocks
NC = T_B // 512            # 2 phase-1 moving chunks per block (pair)
CH = NB * NC               # 8 512-token chunks
NTS = T_B // 128           # 8 phase-2 stationary t-subs per block
ND2 = D_IN // 512          # 4 phase-2 output column chunks (2 pairs)

_nc_cache = None


def _body(nc, tc, pool, xpool, wpool, psum, views):
    f32, bf16 = mybir.dt.float32, mybir.dt.bfloat16
    xT_d, g_d, dn_d, up_d, o_d = views
    Silu = mybir.ActivationFunctionType.Silu

    def xchunk(q):
        return xT_d.ap()[:, q*NK*512:(q+1)*NK*512].rearrange(
            "p (k c) -> p k c", c=512)

    def jtile(t_d, j):
        return t_d.ap()[j*128:(j+1)*128, :].rearrange("p (k c) -> p k c", c=128)

    def dctile(dc):
        return up_d.ap()[dc*128:(dc+1)*128, :].rearrange(
            "p (j c) -> p j c", c=512)

    gts, dts, uts = [], [], []
    for b in range(NB):
        xt = xpool.tile([128, NC, NK, 512], bf16, tag="x")
        gdt0 = None
        if b == 0:
            # startup: interleave j=0 weight quads with x chunk-0 quads so
            # the first k-chains can begin after ~1/4 of the data
            gt0 = pool.tile([128, NK, 128], bf16, tag="g0", name="gt0")
            dt0 = pool.tile([128, NK, 128], bf16, tag="d0", name="dt0")
            gdt0 = (gt0, dt0)
            gv0, dv0 = jtile(g_d, 0), jtile(dn_d, 0)
            xv0, xv1 = xchunk(0), xchunk(1)
            for q in range(4):
                ksl = slice(q*4, (q+1)*4)
                nc.sync.dma_start(gt0[:, ksl], gv0[:, ksl])
                nc.sync.dma_start(xt[:, 0, ksl], xv0[:, ksl])
                nc.sync.dma_start(xt[:, 1, ksl], xv1[:, ksl])
                nc.sync.dma_start(dt0[:, ksl], dv0[:, ksl])
        else:
            for c in range(NC):
                nc.sync.dma_start(xt[:, c], xchunk(b*NC + c))
        hts = []
        for j in range(NJ):
            if b == 0:
                if j == 0:
                    gt, dt = gdt0
                else:
                    gt = pool.tile([128, NK, 128], bf16, tag=f"g{j}",
                                   name=f"gt{j}")
                    dt = pool.tile([128, NK, 128], bf16, tag=f"d{j}",
                                   name=f"dt{j}")
                    nc.sync.dma_start(gt[:], jtile(g_d, j))
                    nc.sync.dma_start(dt[:], jtile(dn_d, j))
                gts.append(gt); dts.append(dt)
            else:
                gt, dt = gts[j], dts[j]
            ht = pool.tile([128, T_B], bf16, tag=f"h{j}")
            pg0 = psum.tile([128, 512], f32, tag="pg0")
            pg1 = psum.tile([128, 512], f32, tag="pg1")
            pd0 = psum.tile([128, 512], f32, tag="pd0")
            pd1 = psum.tile([128, 512], f32, tag="pd1")
            for k in range(NK):
                nc.tensor.matmul(pg0[:], gt[:, k, :], xt[:, 0, k, :],
                                 start=(k == 0), stop=(k == NK-1))
                nc.tensor.matmul(pg1[:], gt[:, k, :], xt[:, 1, k, :],
                                 start=(k == 0), stop=(k == NK-1))
            tmp0 = wpool.tile([128, 512], f32, tag="silu0")
            tmp1 = wpool.tile([128, 512], f32, tag="silu1")
            nc.scalar.activation(tmp0[:], pg0[:], Silu)
            nc.scalar.activation(tmp1[:], pg1[:], Silu)
            for k in range(NK):
                nc.tensor.matmul(pd0[:], dt[:, k, :], xt[:, 0, k, :],
                                 start=(k == 0), stop=(k == NK-1))
                nc.tensor.matmul(pd1[:], dt[:, k, :], xt[:, 1, k, :],
                                 start=(k == 0), stop=(k == NK-1))
            nc.vector.tensor_mul(ht[:, 0:512], tmp0[:], pd0[:])
            nc.vector.tensor_mul(ht[:, 512:T_B], tmp1[:], pd1[:])
            hts.append(ht)
        if b == 0:
            for dc in range(ND2):
                ut = pool.tile([128, NJ, 512], bf16, tag=f"u{dc}", name=f"ut{dc}")
                nc.sync.dma_start(ut[:], dctile(dc))
                uts.append(ut)
        for dp in range(ND2 // 2):
            ut0, ut1 = uts[2*dp], uts[2*dp + 1]
            for ts in range(NTS):
                po0 = psum.tile([128, 512], f32, tag="po0")
                po1 = psum.tile([128, 512], f32, tag="po1")
                for j in range(NJ):
                    hsl = hts[j][:, ts*128:(ts+1)*128]
                    nc.tensor.matmul(po0[:], hsl, ut0[:, j, :],
                                     start=(j == 0), stop=(j == NJ-1))
                    nc.tensor.matmul(po1[:], hsl, ut1[:, j, :],
                                     start=(j == 0), stop=(j == NJ-1))
                ot0 = wpool.tile([128, 512], bf16, tag="ot0")
                ot1 = wpool.tile([128, 512], bf16, tag="ot1")
                nc.any.tensor_copy(ot0[:], po0[:])
                nc.any.tensor_copy(ot1[:], po1[:])
                r0 = b*T_B + ts*128
                nc.sync.dma_start(
                    o_d.ap()[r0:r0+128, (2*dp)*512:(2*dp+1)*512], ot0[:])
                nc.sync.dma_start(
                    o_d.ap()[r0:r0+128, (2*dp+1)*512:(2*dp+2)*512], ot1[:])


def _build(reps=1, timing=False):
    f32, bf16 = mybir.dt.float32, mybir.dt.bfloat16
    nc = bacc.Bacc("TRN2", target_bir_lowering=False, debug=False, num_devices=E)
    kin = "Internal" if timing else "ExternalInput"
    kout = "Internal" if timing else "ExternalOutput"
    xT_d = nc.dram_tensor("xT", [128, CH*NK*512], bf16, kind=kin)
    g_d = nc.dram_tensor("g", [NJ*128, NK*128], bf16, kind=kin)
    dn_d = nc.dram_tensor("dn", [NJ*128, NK*128], bf16, kind=kin)
    up_d = nc.dram_tensor("up", [ND2*128, NJ*512], bf16, kind=kin)
    o_d = nc.dram_tensor("o", [T, D_IN], bf16, kind=kout)
    if timing:
        tin_d = nc.dram_tensor("tin", [1, 64], f32, kind="ExternalInput")
        tout_d = nc.dram_tensor("tout", [1, 64], f32, kind="ExternalOutput")
    views = (xT_d, g_d, dn_d, up_d, o_d)

    with tile.TileContext(nc) as tc:
        with tc.tile_pool(name="sb", bufs=1) as pool, \
             tc.tile_pool(name="xb", bufs=1) as xpool, \
             tc.tile_pool(name="ws", bufs=2) as wpool, \
             tc.tile_pool(name="ps", bufs=1, space="PSUM") as psum1, \
             tc.tile_pool(name="ps2", bufs=2, space="PSUM") as psum2:
            class PS:
                @staticmethod
                def tile(shape, dt, tag):
                    if tag.startswith("po"):
                        return psum2.tile(shape, dt, tag=tag, name=tag)
                    return psum1.tile(shape, dt, tag=tag, name=tag)
            if timing:
                tt = wpool.tile([1, 64], f32, tag="tt")
                nc.sync.dma_start(tt[:], tin_d.ap()[:, :])
            if reps == 1:
                _body(nc, tc, pool, xpool, wpool, PS, views)
            else:
                with tc.For_i(0, reps):
                    _body(nc, tc, pool, xpool, wpool, PS, views)
            if timing:
                nc.sync.dma_start(tout_d.ap()[:, :], tt[:])
    nc.compile()
    return nc


def _get_nc():
    global _nc_cache
    if _nc_cache is None:
        _nc_cache = _build()
    return _nc_cache


def _pack(x_e, g_e, d_e, u_e):
    bf = ml_dtypes.bfloat16
    xT = np.ascontiguousarray(x_e.T).astype(bf)            # [D_IN, T]
    xP = xT.reshape(NK, 128, CH, 512).transpose(1, 2, 0, 3) \
           .reshape(128, CH*NK*512)
    gP = g_e.astype(bf).reshape(NK, 128, NJ, 128) \
            .transpose(2, 1, 0, 3).reshape(NJ*128, NK*128)
    dP = d_e.astype(bf).reshape(NK, 128, NJ, 128) \
            .transpose(2, 1, 0, 3).reshape(NJ*128, NK*128)
    uP = u_e.astype(bf).reshape(NJ, 128, ND2, 512) \
            .transpose(2, 1, 0, 3).reshape(ND2*128, NJ*512)
    return {"xT": np.ascontiguousarray(xP), "g": np.ascontiguousarray(gP),
            "dn": np.ascontiguousarray(dP), "up": np.ascontiguousarray(uP)}


def _make_in_maps(x, gate_proj, down_proj, up_proj):
    return [_pack(x[e], gate_proj[e], down_proj[e], up_proj[e])
            for e in range(E)]


def kernel(x, gate_proj, down_proj, up_proj, tokens_per_expert):
    x = np.asarray(x, dtype=np.float32)
    gate_proj = np.asarray(gate_proj, dtype=np.float32)
    down_proj = np.asarray(down_proj, dtype=np.float32)
    up_proj = np.asarray(up_proj, dtype=np.float32)
    nc = _get_nc()
    in_maps = _make_in_maps(x, gate_proj, down_proj, up_proj)
    res = bass_utils.run_bass_kernel_spmd(nc, in_maps, list(range(E)))
    return np.stack([res.results[e]["o"].astype(np.float32)
                     for e in range(E)], axis=0)

